# revision 82
# baseline (speedup 1.0000x reference)
"""Biaffine scorer kernel for Trainium2 (Bass/Tile), data-parallel over batch
across 8 NeuronCores — bf16 pipeline, streaming schedule (v6, ~48us vs the
~51us v3 baseline; measured 45.9us in a cooler device regime).

Key structural points (evolved from the v3 c-group pipeline):

  - cmat input halved: only rows 0..127 are loaded. Rows 128..255 satisfy
    cmat[x+128, o, y] = cmat[x, o, y-128] for y>=128 and = 0 for y<128 once
    cls_b is folded into the ut matrix (ones x ones position) so that
    wproj[0] = 0 (width_table padding row). The derived half is rebuilt
    on-device (GpSimd memset + 2 strided DVE shift-copies); the o>=4 shift
    is emitted mid-pipeline so the DVE FIFO never blocks early evacuations
    on the late cmat-B transfer.
  - every DMA costs ~0.6-0.7us of issue time on its HWDGE ring's sequencer
    (DIRECT2D), so weights ride in one packed transfer (tw|bias|hw|ut with
    bias bitcast into bf16 columns), state is split into contiguous
    halves, and output goes out in 2-3 column pieces per 128-row tile.
    Issuing DMAs from ACT/GpSimd serializes against their compute, so
    everything stays on the SP ring except the very last output pieces.
  - pair-1 projection matmuls are interleaved into pair-0's finals c-groups
    (one per finals tile), removing the serial proj block between pairs.
  - PE warm-up dummies bridge input-wait gaps so HAM stays at K=8/8
    (2.4GHz); evacuations rotate DVE-fused / ACT-copy+GpSimd-add (~1:1:0.5
    engine balance), with the kernel-tail chunks kept off slow GpSimd.
  - PSUM: 2 proj banks + 2 tut banks (double-wide [121,1024] psum, one
    copy per o-pair) + 4 finals banks in rotation.

The remaining time is pinned by (a) the serial input stream (~3.5MB at
~310GB/s effective incl. ramps), (b) PSUM-egress work: only DVE+ACT can
read PSUM, and the 40 finals chunks + 10 tut double-copies + leakys are
~40us of engine time across the two, (c) ~8us of fixed preamble/drain
inside the measured window.
"""

import numpy as np
import ml_dtypes

import concourse.bass as bass
import concourse.bacc as bacc
import concourse.tile as tile
from concourse import mybir
from concourse.bass_utils import run_bass_kernel_spmd

# problem shape (hardcoded per harness contract)
B, S, H = 32, 255, 1024
BS, WD, O = 120, 20, 10
SP = 256            # padded S
SP2 = 2 * SP        # paired moving dim
NW = SP * O         # 2560
KT = H // 128       # 8
NCORES = 8
BPC = B // NCORES   # 4 batch items per core
NP = BPC // 2       # 2 pairs per core
BSE = BS + 1        # 121
UTW = O * BSE + 6   # 1216 (pad to keep 4B-aligned rows)

F32 = mybir.dt.float32
F16 = mybir.dt.bfloat16
BF16NP = ml_dtypes.bfloat16

_CACHE: dict = {}


def _emit(tc, d):
    """Emit the per-core program. d: dict of DRAM APs."""
    from contextlib import ExitStack

    nc = tc.nc
    AF = mybir.ActivationFunctionType

    with ExitStack() as ctx:
        const = ctx.enter_context(tc.tile_pool(name="const", bufs=1))
        st_pool = ctx.enter_context(tc.tile_pool(name="st", bufs=3))
        ht_pool = ctx.enter_context(tc.tile_pool(name="ht", bufs=4))
        tut_pool = ctx.enter_context(tc.tile_pool(name="tut", bufs=2))
        out_pool = ctx.enter_context(tc.tile_pool(name="outp", bufs=2))
        pp_ht = ctx.enter_context(tc.tile_pool(name="pp_ht", bufs=2, space="PSUM"))
        pp_u = ctx.enter_context(tc.tile_pool(name="pp_u", bufs=1, space="PSUM"))
        pp_s = ctx.enter_context(tc.tile_pool(name="pp_s", bufs=4, space="PSUM"))

        # ---- persistent constants + stateT, in consumption order on the SP
        # HWDGE ring (each dma_start costs ~0.6us of ring issue time, so
        # weights ride in ONE packed transfer): weights feed pair-0 proj
        # immediately; cmat lands right when the first finals evacuate;
        # pair-1 state streams last ----
        WPK = 2 * KT * BSE + 4 + UTW   # 3156
        WA = KT * BSE + 4              # tw | bias  (first transfer)
        sb_wp = const.tile([128, WPK], F16)
        sb_tw = sb_wp[:, 0:KT * BSE]
        sb_bias = sb_wp[0:BSE, KT * BSE:WA].bitcast(F32)
        sb_hw = sb_wp[:, WA:WA + KT * BSE]
        sb_ut = sb_wp[0:BSE, WA + KT * BSE:WPK]
        # combined cmat tile: cols 0:NW = loaded rows 0:128; NW:2NW = derived
        # rows 128:256 — one 3D AP then covers both x-halves of a batch item
        sb_call = const.tile([128, 2 * NW], F16)
        stq0 = [
            st_pool.tile([128, 2048], F16, name=f"stq0_{h}", tag="stq0")
            for h in range(2)
        ]
        stq1 = st_pool.tile([128, KT * SP2], F16, name="stq1", tag="stq1")
        loads = [
            (sb_wp[:, 0:WA], d["wpack"][:, 0:WA]),
            (sb_call[:, 0:1024], d["cmat"][:, 0:1024]),
            (stq0[0][:], d["st0"][0]),
            (stq0[1][:], d["st0"][1]),
            (sb_wp[:, WA:WPK], d["wpack"][:, WA:WPK]),
            (stq1[:, 0:2048], d["st1"][0]),
        ]
        for dst, src_ in loads:
            nc.sync.dma_start(dst, src_)
        # the last two inputs ride the ACT HWDGE ring, which streams in
        # parallel with the SP ring; ACT's sequencer is idle until the first
        # leaky (~15us), so the two ~0.65us issue slots are free
        nc.scalar.dma_start(sb_call[:, 1024:NW], d["cmat"][:, 1024:NW])
        nc.scalar.dma_start(stq1[:, 2048:4096], d["st1"][1])

        # ---- PE warm-up: ~3.4us of continuous busy to leave the 1.2GHz
        # p-state; depends only on a DVE memset so it runs under the DMA head.
        # Dummies write into the finals psum pool (pp_s) so they never touch
        # the single-slot projection accumulator.
        scratch = const.tile([128, 512], F16)
        nc.vector.memset(scratch[:], 0)
        ndum = [0]

        def dummy():
            ps_d = pp_s.tile([128, 512], F32, name=f"dmy_{ndum[0]}", tag="ps_s")
            ndum[0] += 1
            nc.tensor.matmul(
                ps_d[:], lhsT=scratch[:, 0:128], rhs=scratch[:],
                start=True, stop=True,
            )

        for i in range(7):
            dummy()

        # ---- derive cmat rows 128:256 in SBUF: zero the y<128 halves, then
        # shift-copy the y>=128 halves from rows 0:128 (see module docstring).
        # The o>=4 shift reads the late-arriving cmat-B transfer, so its DVE
        # op is emitted later (inside emit_pair, before group c=3 needs it) —
        # otherwise it blocks every earlier evac in the DVE FIFO.
        c1v = sb_call[:, NW:2 * NW].rearrange("p (o y) -> p o y", o=O)
        cAv = sb_call[:, 0:1024].rearrange("p (o y) -> p o y", o=4)
        cBv = sb_call[:, 1024:NW].rearrange("p (o y) -> p o y", o=6)
        nc.gpsimd.memset(c1v[:, :, 0:128], 0)
        nc.vector.tensor_scalar_add(c1v[:, 0:4, 128:256], cAv[:, :, 0:128], 0.0)

        def shiftB():
            nc.vector.tensor_scalar_add(
                c1v[:, 4:10, 128:256], cBv[:, :, 0:128], 0.0)

        # ---- projection helpers (lazy psum per (pair, which)) ----
        proj_ps: dict = {}
        proj_hv: dict = {}

        def proj_mm(p, which, kt):
            key = (p, which)
            if key not in proj_ps:
                proj_ps[key] = pp_ht.tile(
                    [BSE, SP2], F32, name=f"ps_p{p}_{which}", tag="ps"
                )
            w = sb_tw if which else sb_hw
            if p == 0:
                st = stq0[kt // 4]
                rhs = st[:, (kt % 4) * SP2:(kt % 4 + 1) * SP2]
            else:
                rhs = stq1[:, kt * SP2:(kt + 1) * SP2]
            nc.tensor.matmul(
                proj_ps[key][:],
                lhsT=w[:, kt * BSE:(kt + 1) * BSE],
                rhs=rhs,
                start=(kt == 0),
                stop=(kt == KT - 1),
            )

        def proj_leaky(p, which):
            bcol = 1 if which else 0
            hv = ht_pool.tile([BSE, SP2], F16, name=f"ht_p{p}_{which}", tag="hv")
            # leaky(psum + bias); row 120: weights col is 0, bias 1 -> 1.0
            nc.scalar.activation(
                hv[:], proj_ps[(p, which)][:], AF.Lrelu,
                bias=sb_bias[:, bcol:bcol + 1], scale=1.0, alpha=0.01,
            )
            proj_hv[(p, which)] = hv

        tut_tiles = {}

        def emit_tut(p, c):
            """tut matmuls + double-wide copy for o = 2c, 2c+1."""
            if p not in tut_tiles:
                tut_tiles[p] = tut_pool.tile(
                    [BSE, O * SP2], F16, name=f"tut_{p}", tag="tut"
                )
            tut = tut_tiles[p]
            t1T = proj_hv[(p, 1)]
            ps_u = pp_u.tile([BSE, 2 * SP2], F32, name=f"ps_u_{p}_{c}", tag="ps_u")
            for half in range(2):
                o = 2 * c + half
                nc.tensor.matmul(
                    ps_u[:, half * SP2:(half + 1) * SP2],
                    lhsT=sb_ut[:, o * BSE:(o + 1) * BSE],
                    rhs=t1T[:],
                    start=True,
                    stop=True,
                )
            tdst = tut[:, 2 * c * SP2:(2 * c + 2) * SP2]
            if c == 0:
                # split copy so the first finals matmul only waits on o=0
                nc.scalar.activation(
                    tut[:, 0:SP2], ps_u[:, 0:SP2], AF.Copy)
                nc.scalar.activation(
                    tut[:, SP2:2 * SP2], ps_u[:, SP2:2 * SP2], AF.Copy)
            elif c == 3:
                nc.vector.tensor_scalar_add(tdst, ps_u[:], 0.0)
            else:
                nc.scalar.activation(tdst, ps_u[:], AF.Copy)

        kevac = 0

        def emit_pair(p, interleave):
            """tut/finals c-group pipeline for pair p; `interleave` is a list
            of callables inserted one per finals psum tile (pair-1 proj MMs).
            Finals psum is double-wide [128, 1024]: one tile holds both
            x-halves (xt) of one batch item (bb); the evacuation is a single
            3D-AP op per (cc, bb), with the cmat add reading the combined
            sb_call tile (xt=0 half from loaded rows, xt=1 from derived)."""
            nonlocal kevac
            h1T = proj_hv[(p, 0)]
            tut = tut_tiles[p]
            sb_out = out_pool.tile(
                [128, 4 * NW], F16, name=f"sb_out_p{p}", tag="sb_out"
            )
            # dst view [r, bb, xt, col] matching the sb_out (bb, xt, col) packing
            dst4 = d["out"][2 * p:2 * p + 2].rearrange(
                "b (xt r) c -> r b xt c", xt=2
            )
            src4 = sb_out[:].rearrange("r (b xt c) -> r b xt c", b=2, xt=2)
            cov = sb_call[:].rearrange("p (xt c) -> p xt c", xt=2)
            ii = 0  # interleave cursor
            for c in range(1, 6):
                if p == 0 and c == 3:
                    shiftB()
                if c < 5:
                    emit_tut(p, c)
                cc = c - 1
                for i in range(4):
                    bb, xt = i // 2, i % 2
                    lo = bb * SP + xt * 128
                    ps_s = pp_s.tile(
                        [128, 512], F32, name=f"ps_s_{p}_{cc}_{i}", tag="ps_s"
                    )
                    for half in range(2):
                        o = 2 * cc + half
                        nc.tensor.matmul(
                            ps_s[:, half * 256:(half + 1) * 256],
                            lhsT=h1T[:, lo:lo + 128],
                            rhs=tut[:, o * SP2 + bb * SP:o * SP2 + bb * SP + SP],
                            start=True,
                            stop=True,
                        )
                    if c >= 2 and ii < len(interleave):
                        interleave[ii]()
                        ii += 1
                    base = (bb * 2 + xt) * NW
                    oc = sb_out[:, base + cc * 512:base + (cc + 1) * 512]
                    co = cov[:, xt, cc * 512:(cc + 1) * 512]
                    if cc == 4 and p == 1:
                        # kernel tail: keep the slow GpSimd add off the
                        # critical path; DVE/ACT drain the last chunks
                        if i < 2:
                            nc.vector.tensor_add(oc, ps_s[:], co)
                        else:
                            nc.scalar.activation(oc, ps_s[:], AF.Copy)
                            nc.vector.tensor_add(oc, oc, co)
                    elif kevac % 3 == 2:
                        nc.scalar.activation(oc, ps_s[:], AF.Copy)
                        nc.gpsimd.tensor_add(oc, oc, co)
                    else:
                        nc.vector.tensor_add(oc, ps_s[:], co)
                    kevac += 1
                    # per-tile output pieces on the SP HWDGE ring; the very
                    # last pieces split across both HWDGE rings so they don't
                    # FIFO-queue behind earlier transfers
                    pieces = ({2: (0, 1536), 4: (1536, NW)}
                              if p == 0 else
                              {0: (0, 512), 2: (512, 1536), 4: (1536, NW)})
                    if cc in pieces:
                        a, bnd = pieces[cc]
                        eng = nc.scalar if (p == 1 and cc == 4 and xt == 1) \
                            else nc.sync
                        eng.dma_start(
                            d["out"][2 * p + bb,
                                     xt * 128:(xt + 1) * 128, a:bnd],
                            sb_out[:, base + a:base + bnd],
                        )

        # ---- pair 0: projections up front (gated by state halves), with
        # dummy matmuls bridging the input-wait gaps so HAM stays warm ----
        for kt in (0, 1):
            proj_mm(0, 1, kt)
        dummy()
        for kt in (2, 3):
            proj_mm(0, 1, kt)
        dummy()
        dummy()
        for kt in (4, 5):
            proj_mm(0, 1, kt)
        dummy()
        for kt in (6, 7):
            proj_mm(0, 1, kt)
        proj_leaky(0, 1)
        for kt in range(KT):
            proj_mm(0, 0, kt)
        proj_leaky(0, 0)
        emit_tut(0, 0)
        dummy()
        dummy()

        # pair-1 proj interleaved into pair-0's finals groups c=2..5
        inter = []
        for kt in range(KT):
            def f(kt=kt):
                proj_mm(1, 1, kt)
                if kt == KT - 1:
                    proj_leaky(1, 1)
            inter.append(f)
        for kt in range(KT):
            def g(kt=kt):
                proj_mm(1, 0, kt)
                if kt == 0:
                    emit_tut(1, 0)
                if kt == KT - 1:
                    proj_leaky(1, 0)
            inter.append(g)

        emit_pair(0, inter)
        emit_pair(1, [])


def build_nc():
    if "nc" in _CACHE:
        return _CACHE["nc"]
    nc = bacc.Bacc(
        "TRN2", target_bir_lowering=False, debug=False, num_devices=NCORES
    )
    d = {}
    WPK = 2 * KT * BSE + 4 + UTW
    d["st0"] = nc.dram_tensor(
        "st0", [2, 128, 2048], F16, kind="ExternalInput"
    ).ap()
    d["st1"] = nc.dram_tensor(
        "st1", [2, 128, 2048], F16, kind="ExternalInput"
    ).ap()
    d["wpack"] = nc.dram_tensor("wpack", [128, WPK], F16, kind="ExternalInput").ap()
    d["cmat"] = nc.dram_tensor("cmat", [128, NW], F16, kind="ExternalInput").ap()
    d["out"] = nc.dram_tensor("out", [BPC, SP, NW], F16, kind="ExternalOutput").ap()

    with tile.TileContext(nc) as tc:
        _emit(tc, d)
    nc.compile()
    _CACHE["nc"] = nc
    return nc


def prep_inputs(inputs):
    """Host-side packing + fp32->bf16 conversion. Returns dict of np arrays
    shared across cores (stateT is full-batch; shard before dispatch)."""
    state = np.asarray(inputs["state"], np.float32)
    head_w = np.asarray(inputs["head_w"], np.float32)
    head_b = np.asarray(inputs["head_b"], np.float32)
    tail_w = np.asarray(inputs["tail_w"], np.float32)
    tail_b = np.asarray(inputs["tail_b"], np.float32)
    U = np.asarray(inputs["U"], np.float32)
    width_table = np.asarray(inputs["width_table"], np.float32)
    cls_w = np.asarray(inputs["cls_w"], np.float32)
    cls_b = np.asarray(inputs["cls_b"], np.float32)

    # stateT paired pack: [B/2, 128, (kt, b01, y)], y zero-padded to 256
    stateT = np.zeros((B, H, SP), np.float32)
    stateT[:, :, :S] = state.transpose(0, 2, 1)
    stateT = stateT.reshape(B // 2, 2, KT, 128, SP).transpose(0, 3, 2, 1, 4)
    stateT = np.ascontiguousarray(
        stateT.reshape(B // 2, 128, KT * SP2).astype(BF16NP)
    )

    hw_sb = np.zeros((128, KT, BSE), np.float32)
    hw_sb[:, :, :BS] = head_w.reshape(KT, 128, BS).transpose(1, 0, 2)
    hw_sb = hw_sb.reshape(128, KT * BSE).astype(BF16NP)
    tw_sb = np.zeros((128, KT, BSE), np.float32)
    tw_sb[:, :, :BS] = tail_w.reshape(KT, 128, BS).transpose(1, 0, 2)
    tw_sb = tw_sb.reshape(128, KT * BSE).astype(BF16NP)

    # ut blocks: [j, o, i] = U[o,i,j]; col 120 = Wt_ext; row 120 += Wh_ext;
    # cls_b folded into [120, o, 120] (multiplied by ones x ones)
    ut = np.zeros((BSE, UTW), np.float32)
    blocks = ut[:, :O * BSE].reshape(BSE, O, BSE)
    blocks[:BS, :, :BS] = U.transpose(2, 0, 1)
    blocks[:, :, BS] = cls_w[:, BS + 1:2 * (BS + 1)].T
    blocks[BS, :, :] += cls_w[:, :BSE]
    blocks[BS, :, BS] += cls_b
    ut = np.ascontiguousarray(ut.astype(BF16NP))

    bias2 = np.zeros((BSE, 2), np.float32)
    bias2[:BS, 0] = head_b
    bias2[BS, 0] = 1.0
    bias2[:BS, 1] = tail_b
    bias2[BS, 1] = 1.0

    # cmat[x, o*256+y] = wproj[pos(x,y), o] for x<128 only; wproj excludes
    # cls_b so wproj[0] = 0 and rows 128:256 are kernel-derived
    pos = np.arange(S)[None, :] - np.arange(S)[:, None] + 1
    pos = pos * (pos > 0)
    posP = np.zeros((SP, SP), np.int64)
    posP[:S, :S] = pos
    wproj = width_table @ cls_w[:, 2 * (BS + 1):].T        # [256, 10], row 0 = 0
    cmat = wproj[posP[:128]]                               # [128, y, o]
    cmat = np.ascontiguousarray(
        cmat.transpose(0, 2, 1).reshape(128, NW).astype(BF16NP)
    )

    # packed weights: tw | bias2-as-bf16-bits | hw | ut (rows 0:121 used)
    WPK = 2 * KT * BSE + 4 + UTW
    WA = KT * BSE + 4
    wpack = np.zeros((128, WPK), BF16NP)
    wpack[:, 0:KT * BSE] = tw_sb
    wpack[:BSE, KT * BSE:WA] = bias2.view(BF16NP)
    wpack[:, WA:WA + KT * BSE] = hw_sb
    wpack[:BSE, WA + KT * BSE:WPK] = ut
    wpack = np.ascontiguousarray(wpack)

    # state as contiguous halves per pair
    sth = np.ascontiguousarray(
        stateT.reshape(B // 2, 128, 2, 2048).transpose(0, 2, 1, 3)
    )  # [B/2, 2, 128, 2048]

    return {
        "st0": sth,
        "st1": sth,
        "wpack": wpack,
        "cmat": cmat,
    }


def run(inputs, trace=False, trace_kwargs=None):
    nc = build_nc()
    full = prep_inputs(inputs)
    shared = {k: v for k, v in full.items() if k not in ("st0", "st1")}
    in_maps = []
    for c in range(NCORES):
        m = dict(shared)
        m["st0"] = np.ascontiguousarray(full["st0"][c * NP])
        m["st1"] = np.ascontiguousarray(full["st1"][c * NP + 1])
        in_maps.append(m)
    res = run_bass_kernel_spmd(
        nc,
        in_maps,
        core_ids=list(range(NCORES)),
        trace=trace,
        **(trace_kwargs or {}),
    )
    out = np.concatenate([r["out"] for r in res.results], axis=0)
    # [B, x(256), (o,y)] bf16 -> [B, x, y, o] fp32, trim padding
    out = out.astype(np.float32).reshape(B, SP, O, SP)
    out = np.ascontiguousarray(out.transpose(0, 1, 3, 2)[:, :S, :S, :])
    return out, res


def kernel(**inputs):
    out, _ = run(inputs, trace=False)
    return out


if __name__ == "__main__":
    build_nc()
    print("build ok")


# revision 84
# speedup vs baseline: 1.1020x; 1.1020x over previous
"""Biaffine scorer kernel for Trainium2 (Bass/Tile), data-parallel over batch
across 8 NeuronCores — bf16 pipeline, streaming schedule (v6, ~48us vs the
~51us v3 baseline; measured 45.9us in a cooler device regime).

Key structural points (evolved from the v3 c-group pipeline):

  - cmat input halved: only rows 0..127 are loaded. Rows 128..255 satisfy
    cmat[x+128, o, y] = cmat[x, o, y-128] for y>=128 and = 0 for y<128 once
    cls_b is folded into the ut matrix (ones x ones position) so that
    wproj[0] = 0 (width_table padding row). The derived half is rebuilt
    on-device (GpSimd memset + 2 strided DVE shift-copies); the o>=4 shift
    is emitted mid-pipeline so the DVE FIFO never blocks early evacuations
    on the late cmat-B transfer.
  - every DMA costs ~0.6-0.7us of issue time on its HWDGE ring's sequencer
    (DIRECT2D), so weights ride in one packed transfer (tw|bias|hw|ut with
    bias bitcast into bf16 columns), state is split into contiguous
    halves, and output goes out in 2-3 column pieces per 128-row tile.
    Issuing DMAs from ACT/GpSimd serializes against their compute, so
    everything stays on the SP ring except the very last output pieces.
  - pair-1 projection matmuls are interleaved into pair-0's finals c-groups
    (one per finals tile), removing the serial proj block between pairs.
  - PE warm-up dummies bridge input-wait gaps so HAM stays at K=8/8
    (2.4GHz); evacuations rotate DVE-fused / ACT-copy+GpSimd-add (~1:1:0.5
    engine balance), with the kernel-tail chunks kept off slow GpSimd.
  - PSUM: 2 proj banks + 2 tut banks (double-wide [121,1024] psum, one
    copy per o-pair) + 4 finals banks in rotation.

The remaining time is pinned by (a) the serial input stream (~3.5MB at
~310GB/s effective incl. ramps), (b) PSUM-egress work: only DVE+ACT can
read PSUM, and the 40 finals chunks + 10 tut double-copies + leakys are
~40us of engine time across the two, (c) ~8us of fixed preamble/drain
inside the measured window.
"""

import numpy as np
import ml_dtypes

import concourse.bass as bass
import concourse.bacc as bacc
import concourse.tile as tile
from concourse import mybir
from concourse.bass_utils import run_bass_kernel_spmd

# problem shape (hardcoded per harness contract)
B, S, H = 32, 255, 1024
BS, WD, O = 120, 20, 10
SP = 256            # padded S
SP2 = 2 * SP        # paired moving dim
NW = SP * O         # 2560
KT = H // 128       # 8
NCORES = 8
BPC = B // NCORES   # 4 batch items per core
NP = BPC // 2       # 2 pairs per core
BSE = BS + 1        # 121
UTW = O * BSE + 6   # 1216 (pad to keep 4B-aligned rows)

F32 = mybir.dt.float32
F16 = mybir.dt.bfloat16
BF16NP = ml_dtypes.bfloat16

_CACHE: dict = {}


def _emit(tc, d):
    """Emit the per-core program. d: dict of DRAM APs."""
    from contextlib import ExitStack

    nc = tc.nc
    AF = mybir.ActivationFunctionType

    with ExitStack() as ctx:
        const = ctx.enter_context(tc.tile_pool(name="const", bufs=1))
        st_pool = ctx.enter_context(tc.tile_pool(name="st", bufs=3))
        ht_pool = ctx.enter_context(tc.tile_pool(name="ht", bufs=4))
        tut_pool = ctx.enter_context(tc.tile_pool(name="tut", bufs=2))
        out_pool = ctx.enter_context(tc.tile_pool(name="outp", bufs=2))
        pp_ht = ctx.enter_context(tc.tile_pool(name="pp_ht", bufs=2, space="PSUM"))
        pp_u = ctx.enter_context(tc.tile_pool(name="pp_u", bufs=1, space="PSUM"))
        pp_s = ctx.enter_context(tc.tile_pool(name="pp_s", bufs=4, space="PSUM"))

        # ---- persistent constants + stateT, in consumption order on the SP
        # HWDGE ring (each dma_start costs ~0.6us of ring issue time, so
        # weights ride in ONE packed transfer): weights feed pair-0 proj
        # immediately; cmat lands right when the first finals evacuate;
        # pair-1 state streams last ----
        WPK = 2 * KT * BSE + 4 + UTW   # 3156
        WA = KT * BSE + 4              # tw | bias  (first transfer)
        sb_wp = const.tile([128, WPK], F16)
        sb_tw = sb_wp[:, 0:KT * BSE]
        sb_bias = sb_wp[0:BSE, KT * BSE:WA].bitcast(F32)
        sb_hw = sb_wp[:, WA:WA + KT * BSE]
        sb_ut = sb_wp[0:BSE, WA + KT * BSE:WPK]
        # combined cmat tile: cols 0:NW = loaded rows 0:128; NW:2NW = derived
        # rows 128:256 — one 3D AP then covers both x-halves of a batch item
        sb_call = const.tile([128, 2 * NW], F16)
        stq0 = [
            st_pool.tile([128, 2048], F16, name=f"stq0_{h}", tag="stq0")
            for h in range(2)
        ]
        stq1 = st_pool.tile([128, KT * SP2], F16, name="stq1", tag="stq1")
        loads = [
            (sb_wp[:, 0:WA], d["wpack"][:, 0:WA]),
            (sb_call[:, 0:1024], d["cmat"][:, 0:1024]),
            (stq0[0][:], d["st0"][0]),
            (stq0[1][:], d["st0"][1]),
            (sb_wp[:, WA:WPK], d["wpack"][:, WA:WPK]),
            (stq1[:, 0:2048], d["st1"][0]),
            (stq1[:, 2048:4096], d["st1"][1]),
            (sb_call[:, 1024:NW], d["cmat"][:, 1024:NW]),
        ]
        for dst, src_ in loads:
            nc.sync.dma_start(dst, src_)

        # ---- PE warm-up: ~3.4us of continuous busy to leave the 1.2GHz
        # p-state; depends only on a DVE memset so it runs under the DMA head.
        # Dummies write into the finals psum pool (pp_s) so they never touch
        # the single-slot projection accumulator.
        scratch = const.tile([128, 512], F16)
        nc.vector.memset(scratch[:], 0)
        ndum = [0]

        def dummy():
            ps_d = pp_s.tile([128, 512], F32, name=f"dmy_{ndum[0]}", tag="ps_s")
            ndum[0] += 1
            nc.tensor.matmul(
                ps_d[:], lhsT=scratch[:, 0:128], rhs=scratch[:],
                start=True, stop=True,
            )

        for i in range(7):
            dummy()

        # ---- derive cmat rows 128:256 in SBUF: zero the y<128 halves, then
        # shift-copy the y>=128 halves from rows 0:128 (see module docstring).
        # The o>=4 shift reads the late-arriving cmat-B transfer, so its DVE
        # op is emitted later (inside emit_pair, before group c=3 needs it) —
        # otherwise it blocks every earlier evac in the DVE FIFO.
        c1v = sb_call[:, NW:2 * NW].rearrange("p (o y) -> p o y", o=O)
        cAv = sb_call[:, 0:1024].rearrange("p (o y) -> p o y", o=4)
        cBv = sb_call[:, 1024:NW].rearrange("p (o y) -> p o y", o=6)
        nc.gpsimd.memset(c1v[:, :, 0:128], 0)
        nc.vector.tensor_scalar_add(c1v[:, 0:4, 128:256], cAv[:, :, 0:128], 0.0)

        def shiftB():
            nc.vector.tensor_scalar_add(
                c1v[:, 4:10, 128:256], cBv[:, :, 0:128], 0.0)

        # ---- projection helpers (lazy psum per (pair, which)) ----
        proj_ps: dict = {}
        proj_hv: dict = {}

        def proj_mm(p, which, kt):
            key = (p, which)
            if key not in proj_ps:
                proj_ps[key] = pp_ht.tile(
                    [BSE, SP2], F32, name=f"ps_p{p}_{which}", tag="ps"
                )
            w = sb_tw if which else sb_hw
            if p == 0:
                st = stq0[kt // 4]
                rhs = st[:, (kt % 4) * SP2:(kt % 4 + 1) * SP2]
            else:
                rhs = stq1[:, kt * SP2:(kt + 1) * SP2]
            nc.tensor.matmul(
                proj_ps[key][:],
                lhsT=w[:, kt * BSE:(kt + 1) * BSE],
                rhs=rhs,
                start=(kt == 0),
                stop=(kt == KT - 1),
            )

        def proj_leaky(p, which):
            bcol = 1 if which else 0
            hv = ht_pool.tile([BSE, SP2], F16, name=f"ht_p{p}_{which}", tag="hv")
            # leaky(psum + bias); row 120: weights col is 0, bias 1 -> 1.0
            nc.scalar.activation(
                hv[:], proj_ps[(p, which)][:], AF.Lrelu,
                bias=sb_bias[:, bcol:bcol + 1], scale=1.0, alpha=0.01,
            )
            proj_hv[(p, which)] = hv

        tut_tiles = {}

        def emit_tut(p, c):
            """tut matmuls + double-wide copy for o = 2c, 2c+1."""
            if p not in tut_tiles:
                tut_tiles[p] = tut_pool.tile(
                    [BSE, O * SP2], F16, name=f"tut_{p}", tag="tut"
                )
            tut = tut_tiles[p]
            t1T = proj_hv[(p, 1)]
            ps_u = pp_u.tile([BSE, 2 * SP2], F32, name=f"ps_u_{p}_{c}", tag="ps_u")
            for half in range(2):
                o = 2 * c + half
                nc.tensor.matmul(
                    ps_u[:, half * SP2:(half + 1) * SP2],
                    lhsT=sb_ut[:, o * BSE:(o + 1) * BSE],
                    rhs=t1T[:],
                    start=True,
                    stop=True,
                )
            tdst = tut[:, 2 * c * SP2:(2 * c + 2) * SP2]
            if c == 0:
                # split copy so the first finals matmul only waits on o=0
                nc.scalar.activation(
                    tut[:, 0:SP2], ps_u[:, 0:SP2], AF.Copy)
                nc.scalar.activation(
                    tut[:, SP2:2 * SP2], ps_u[:, SP2:2 * SP2], AF.Copy)
            elif c == 3:
                nc.vector.tensor_scalar_add(tdst, ps_u[:], 0.0)
            else:
                nc.scalar.activation(tdst, ps_u[:], AF.Copy)

        kevac = 0

        def emit_pair(p, interleave):
            """tut/finals c-group pipeline for pair p; `interleave` is a list
            of callables inserted one per finals psum tile (pair-1 proj MMs).
            Finals psum is double-wide [128, 1024]: one tile holds both
            x-halves (xt) of one batch item (bb); the evacuation is a single
            3D-AP op per (cc, bb), with the cmat add reading the combined
            sb_call tile (xt=0 half from loaded rows, xt=1 from derived)."""
            nonlocal kevac
            h1T = proj_hv[(p, 0)]
            tut = tut_tiles[p]
            sb_out = out_pool.tile(
                [128, 4 * NW], F16, name=f"sb_out_p{p}", tag="sb_out"
            )
            # dst view [r, bb, xt, col] matching the sb_out (bb, xt, col) packing
            dst4 = d["out"][2 * p:2 * p + 2].rearrange(
                "b (xt r) c -> r b xt c", xt=2
            )
            src4 = sb_out[:].rearrange("r (b xt c) -> r b xt c", b=2, xt=2)
            cov = sb_call[:].rearrange("p (xt c) -> p xt c", xt=2)
            ii = 0  # interleave cursor
            for c in range(1, 6):
                if p == 0 and c == 3:
                    shiftB()
                if c < 5:
                    emit_tut(p, c)
                cc = c - 1
                for i in range(4):
                    bb, xt = i // 2, i % 2
                    lo = bb * SP + xt * 128
                    ps_s = pp_s.tile(
                        [128, 512], F32, name=f"ps_s_{p}_{cc}_{i}", tag="ps_s"
                    )
                    for half in range(2):
                        o = 2 * cc + half
                        nc.tensor.matmul(
                            ps_s[:, half * 256:(half + 1) * 256],
                            lhsT=h1T[:, lo:lo + 128],
                            rhs=tut[:, o * SP2 + bb * SP:o * SP2 + bb * SP + SP],
                            start=True,
                            stop=True,
                        )
                    if c >= 2 and ii < len(interleave):
                        interleave[ii]()
                        ii += 1
                    base = (bb * 2 + xt) * NW
                    oc = sb_out[:, base + cc * 512:base + (cc + 1) * 512]
                    co = cov[:, xt, cc * 512:(cc + 1) * 512]
                    if cc == 4 and p == 1:
                        # kernel tail: keep the slow GpSimd add off the
                        # critical path; DVE/ACT drain the last chunks
                        if i < 2:
                            nc.vector.tensor_add(oc, ps_s[:], co)
                        else:
                            nc.scalar.activation(oc, ps_s[:], AF.Copy)
                            nc.vector.tensor_add(oc, oc, co)
                    elif kevac % 3 == 2:
                        nc.scalar.activation(oc, ps_s[:], AF.Copy)
                        if xt == 1:
                            # derived-cmat rows are zero for y<128: add only
                            # the y>=128 half of each o block (half the work)
                            och = oc.rearrange(
                                "p (o y) -> p o y", o=2)[:, :, 128:256]
                            coh = co.rearrange(
                                "p (o y) -> p o y", o=2)[:, :, 128:256]
                            nc.gpsimd.tensor_add(och, och, coh)
                        else:
                            nc.gpsimd.tensor_add(oc, oc, co)
                    else:
                        nc.vector.tensor_add(oc, ps_s[:], co)
                    kevac += 1
                    # per-tile output pieces on the SP HWDGE ring; the very
                    # last pieces split across both HWDGE rings so they don't
                    # FIFO-queue behind earlier transfers
                    pieces = ({2: (0, 1536), 4: (1536, NW)}
                              if p == 0 else
                              {0: (0, 512), 2: (512, 1536), 4: (1536, NW)})
                    if cc in pieces:
                        a, bnd = pieces[cc]
                        eng = nc.scalar if (p == 1 and cc == 4 and xt == 1) \
                            else nc.sync
                        eng.dma_start(
                            d["out"][2 * p + bb,
                                     xt * 128:(xt + 1) * 128, a:bnd],
                            sb_out[:, base + a:base + bnd],
                        )

        # ---- pair 0: projections up front (gated by state halves), with
        # dummy matmuls bridging the input-wait gaps so HAM stays warm ----
        for kt in (0, 1):
            proj_mm(0, 1, kt)
        dummy()
        for kt in (2, 3):
            proj_mm(0, 1, kt)
        dummy()
        dummy()
        for kt in (4, 5):
            proj_mm(0, 1, kt)
        dummy()
        for kt in (6, 7):
            proj_mm(0, 1, kt)
        proj_leaky(0, 1)
        for kt in range(KT):
            proj_mm(0, 0, kt)
        proj_leaky(0, 0)
        emit_tut(0, 0)
        dummy()
        dummy()

        # pair-1 proj interleaved into pair-0's finals groups c=2..5
        inter = []
        for kt in range(KT):
            def f(kt=kt):
                proj_mm(1, 1, kt)
                if kt == KT - 1:
                    proj_leaky(1, 1)
            inter.append(f)
        for kt in range(KT):
            def g(kt=kt):
                proj_mm(1, 0, kt)
                if kt == 0:
                    emit_tut(1, 0)
                if kt == KT - 1:
                    proj_leaky(1, 0)
            inter.append(g)

        emit_pair(0, inter)
        emit_pair(1, [])


def build_nc():
    if "nc" in _CACHE:
        return _CACHE["nc"]
    nc = bacc.Bacc(
        "TRN2", target_bir_lowering=False, debug=False, num_devices=NCORES
    )
    d = {}
    WPK = 2 * KT * BSE + 4 + UTW
    d["st0"] = nc.dram_tensor(
        "st0", [2, 128, 2048], F16, kind="ExternalInput"
    ).ap()
    d["st1"] = nc.dram_tensor(
        "st1", [2, 128, 2048], F16, kind="ExternalInput"
    ).ap()
    d["wpack"] = nc.dram_tensor("wpack", [128, WPK], F16, kind="ExternalInput").ap()
    d["cmat"] = nc.dram_tensor("cmat", [128, NW], F16, kind="ExternalInput").ap()
    d["out"] = nc.dram_tensor("out", [BPC, SP, NW], F16, kind="ExternalOutput").ap()

    with tile.TileContext(nc) as tc:
        _emit(tc, d)
    nc.compile()
    _CACHE["nc"] = nc
    return nc


def prep_inputs(inputs):
    """Host-side packing + fp32->bf16 conversion. Returns dict of np arrays
    shared across cores (stateT is full-batch; shard before dispatch)."""
    state = np.asarray(inputs["state"], np.float32)
    head_w = np.asarray(inputs["head_w"], np.float32)
    head_b = np.asarray(inputs["head_b"], np.float32)
    tail_w = np.asarray(inputs["tail_w"], np.float32)
    tail_b = np.asarray(inputs["tail_b"], np.float32)
    U = np.asarray(inputs["U"], np.float32)
    width_table = np.asarray(inputs["width_table"], np.float32)
    cls_w = np.asarray(inputs["cls_w"], np.float32)
    cls_b = np.asarray(inputs["cls_b"], np.float32)

    # stateT paired pack: [B/2, 128, (kt, b01, y)], y zero-padded to 256
    stateT = np.zeros((B, H, SP), np.float32)
    stateT[:, :, :S] = state.transpose(0, 2, 1)
    stateT = stateT.reshape(B // 2, 2, KT, 128, SP).transpose(0, 3, 2, 1, 4)
    stateT = np.ascontiguousarray(
        stateT.reshape(B // 2, 128, KT * SP2).astype(BF16NP)
    )

    hw_sb = np.zeros((128, KT, BSE), np.float32)
    hw_sb[:, :, :BS] = head_w.reshape(KT, 128, BS).transpose(1, 0, 2)
    hw_sb = hw_sb.reshape(128, KT * BSE).astype(BF16NP)
    tw_sb = np.zeros((128, KT, BSE), np.float32)
    tw_sb[:, :, :BS] = tail_w.reshape(KT, 128, BS).transpose(1, 0, 2)
    tw_sb = tw_sb.reshape(128, KT * BSE).astype(BF16NP)

    # ut blocks: [j, o, i] = U[o,i,j]; col 120 = Wt_ext; row 120 += Wh_ext;
    # cls_b folded into [120, o, 120] (multiplied by ones x ones)
    ut = np.zeros((BSE, UTW), np.float32)
    blocks = ut[:, :O * BSE].reshape(BSE, O, BSE)
    blocks[:BS, :, :BS] = U.transpose(2, 0, 1)
    blocks[:, :, BS] = cls_w[:, BS + 1:2 * (BS + 1)].T
    blocks[BS, :, :] += cls_w[:, :BSE]
    blocks[BS, :, BS] += cls_b
    ut = np.ascontiguousarray(ut.astype(BF16NP))

    bias2 = np.zeros((BSE, 2), np.float32)
    bias2[:BS, 0] = head_b
    bias2[BS, 0] = 1.0
    bias2[:BS, 1] = tail_b
    bias2[BS, 1] = 1.0

    # cmat[x, o*256+y] = wproj[pos(x,y), o] for x<128 only; wproj excludes
    # cls_b so wproj[0] = 0 and rows 128:256 are kernel-derived
    pos = np.arange(S)[None, :] - np.arange(S)[:, None] + 1
    pos = pos * (pos > 0)
    posP = np.zeros((SP, SP), np.int64)
    posP[:S, :S] = pos
    wproj = width_table @ cls_w[:, 2 * (BS + 1):].T        # [256, 10], row 0 = 0
    cmat = wproj[posP[:128]]                               # [128, y, o]
    cmat = np.ascontiguousarray(
        cmat.transpose(0, 2, 1).reshape(128, NW).astype(BF16NP)
    )

    # packed weights: tw | bias2-as-bf16-bits | hw | ut (rows 0:121 used)
    WPK = 2 * KT * BSE + 4 + UTW
    WA = KT * BSE + 4
    wpack = np.zeros((128, WPK), BF16NP)
    wpack[:, 0:KT * BSE] = tw_sb
    wpack[:BSE, KT * BSE:WA] = bias2.view(BF16NP)
    wpack[:, WA:WA + KT * BSE] = hw_sb
    wpack[:BSE, WA + KT * BSE:WPK] = ut
    wpack = np.ascontiguousarray(wpack)

    # state as contiguous halves per pair
    sth = np.ascontiguousarray(
        stateT.reshape(B // 2, 128, 2, 2048).transpose(0, 2, 1, 3)
    )  # [B/2, 2, 128, 2048]

    return {
        "st0": sth,
        "st1": sth,
        "wpack": wpack,
        "cmat": cmat,
    }


def run(inputs, trace=False, trace_kwargs=None):
    nc = build_nc()
    full = prep_inputs(inputs)
    shared = {k: v for k, v in full.items() if k not in ("st0", "st1")}
    in_maps = []
    for c in range(NCORES):
        m = dict(shared)
        m["st0"] = np.ascontiguousarray(full["st0"][c * NP])
        m["st1"] = np.ascontiguousarray(full["st1"][c * NP + 1])
        in_maps.append(m)
    res = run_bass_kernel_spmd(
        nc,
        in_maps,
        core_ids=list(range(NCORES)),
        trace=trace,
        **(trace_kwargs or {}),
    )
    out = np.concatenate([r["out"] for r in res.results], axis=0)
    # [B, x(256), (o,y)] bf16 -> [B, x, y, o] fp32, trim padding
    out = out.astype(np.float32).reshape(B, SP, O, SP)
    out = np.ascontiguousarray(out.transpose(0, 1, 3, 2)[:, :S, :S, :])
    return out, res


def kernel(**inputs):
    out, _ = run(inputs, trace=False)
    return out


if __name__ == "__main__":
    build_nc()
    print("build ok")
